# revision 19
# baseline (speedup 1.0000x reference)
"""FP8Linear on 8 Trainium2 NeuronCores (Bass/Tile, SPMD).

Reference math (per nn_FP8Linear):
    amax_x = max|x|, amax_w = max|w|               (global)
    x_scale = amax_x / C,  w_scale = max(amax_x, amax_w) / C,  C = fp32(448*0.8)
    out = dequant(e4m3fn(x/x_scale)) @ dequant(e4m3fn(w/w_scale)).T + bias

Sharding: tokens (B*S=8192) split across 8 cores (1024 each); weight
replicated. Inputs are staged pre-transposed ([Din, tok] / [Din, Dout]) so the
contraction dim lands on SBUF partitions with no on-device transposes.

Device per core:
  - local abs-max of its x shard + its 1/8 row-slice of w (extra 2MB input)
  - AllReduce(max) of [amax_x, amax_w] across the 8 cores (ncfw; a dummy
    warm-up collective is issued at t=0 to absorb channel-setup latency)
  - scales: s = amax*(1/C); quantize t = x * rn(1/(2s)) -> TRN fp8e4.
    TRN fp8e4 max is +-240 (not e4m3fn's 448); quantizing x/(2s) instead of
    x/s keeps values <= 179.2 and exactly halves the e4m3fn grid, which the
    final output scale 4*s_x*s_w undoes.
  - fp8 matmuls accumulating over Din in PSUM; out = psum*(4 s_x s_w) + bias
"""
import numpy as np
from contextlib import ExitStack

import concourse.bacc as bacc
import concourse.bass as bass
import concourse.bass_isa as bass_isa
import concourse.tile as tile
from concourse import mybir
from concourse.bass_utils import run_bass_kernel_spmd

N_CORES = 8
B, S, DIN, DOUT = 4, 2048, 2048, 2048
TOK = B * S                  # 8192 tokens
TSH = TOK // N_CORES         # 1024 tokens per core
KT = DIN // 128              # 16 k-tiles
NT = DOUT // 512             # 4 n-groups
MT = TSH // 128              # 8 token-tiles

C_F64 = 448.0 * 0.8          # what jax sees before fp32 demotion
RC = float(np.float32(1.0 / np.float64(np.float32(C_F64))))  # rn(1/C)

USE_DOUBLE_ROW = True        # fp8 DoubleRow perf mode (2x PE, ~1e-4 extra err)
WT_BUFS = 9                  # fp32 w-tile streaming slots (8KB/partition each)
ENABLE_LDW_OPT = False       # walrus rejects ldw-opt on bass-emitted Ldweights


def _patch_ldw_opt():
    """The walrus cmdline in bass_utils hardcodes --enable-ldw-opt=false;
    rewrite it so repeated-stationary matmuls skip the weight reload."""
    import concourse.bass_utils as bu
    if getattr(bu, "_ldw_opt_patched", False):
        return
    orig = bu.run_command

    def patched(cmd, *a, **kw):
        if isinstance(cmd, list):
            cmd = ["--enable-ldw-opt=true" if c == "--enable-ldw-opt=false"
                   else c for c in cmd]
        return orig(cmd, *a, **kw)

    bu.run_command = patched
    bu._ldw_opt_patched = True


if ENABLE_LDW_OPT:
    _patch_ldw_opt()

F32 = mybir.dt.float32
F8 = mybir.dt.float8e4

_built = None


def _build():
    global _built
    if _built is not None:
        return _built
    nc = bacc.Bacc("TRN2", target_bir_lowering=False, num_devices=N_CORES)

    xt_d = nc.dram_tensor("xt", [DIN, TSH], F32, kind="ExternalInput")
    wt_d = nc.dram_tensor("wt", [DIN, DOUT], F32, kind="ExternalInput")
    wsl_d = nc.dram_tensor("wsl", [128, DIN * DOUT // N_CORES // 128], F32,
                           kind="ExternalInput")   # [128, 4096]
    bias_d = nc.dram_tensor("bias", [1, DOUT], F32, kind="ExternalInput")
    out_d = nc.dram_tensor("out", [TSH, DOUT], F32, kind="ExternalOutput")

    WSLF = DIN * DOUT // N_CORES // 128  # 4096

    with tile.TileContext(nc) as tc, ExitStack() as ctx:
        pool = ctx.enter_context(tc.tile_pool(name="pool", bufs=1))
        opool = ctx.enter_context(tc.tile_pool(name="opool", bufs=4))
        psum = ctx.enter_context(tc.tile_pool(name="psum", bufs=8, space="PSUM"))
        dram = ctx.enter_context(tc.tile_pool(name="dram", bufs=1, space="DRAM"))
        wslctx = ExitStack()
        wslpool = wslctx.enter_context(tc.tile_pool(name="wslpool", bufs=1))

        # ---- dummy collective at t=0: starts ncfw channel setup early
        warm = pool.tile([1, 8], F32)
        nc.vector.memset(warm[:], 0.0)
        cw_in = dram.tile([1, 8], F32)
        cw_out = dram.tile([1, 8], F32)
        with nc.named_scope("cc_warm"):
            nc.sync.dma_start(cw_in[:], warm[:])
            nc.gpsimd.collective_compute(
                "AllReduce", mybir.AluOpType.max,
                replica_groups=[list(range(N_CORES))],
                ins=[cw_in[:].opt()], outs=[cw_out[:].opt()])

        # ---- input DMA: x shard (transposed) in 4 chunks, then wsl, bias
        xt_sb = pool.tile([128, KT, TSH], F32)
        xt_view = xt_d[:].rearrange("(k p) c -> p k c", p=128)
        with nc.named_scope("dma_x"):
            for c in range(4):
                nc.sync.dma_start(xt_sb[:, 4 * c:4 * c + 4, :],
                                  xt_view[:, 4 * c:4 * c + 4, :])
        wsl_sb = wslpool.tile([128, WSLF], F32)
        bias_bc = pool.tile([128, DOUT], F32)
        with nc.named_scope("dma_wsl"):
            nc.sync.dma_start(wsl_sb[:], wsl_d[:])

        # ---- local abs-max
        red = pool.tile([128, 8], F32)
        nc.vector.memset(red[:], 0.0)
        with nc.named_scope("amax"):
            for c in range(4):
                nc.vector.tensor_reduce(red[:, c:c + 1],
                                        xt_sb[:, 4 * c:4 * c + 4, :],
                                        mybir.AxisListType.XY,
                                        mybir.AluOpType.max,
                                        apply_absolute_value=True)
            nc.vector.tensor_reduce(red[:, 4:5], wsl_sb[:],
                                    mybir.AxisListType.X,
                                    mybir.AluOpType.max,
                                    apply_absolute_value=True)
            am2 = pool.tile([128, 2], F32)
            nc.vector.tensor_reduce(am2[:, 0:1], red[:, 0:4],
                                    mybir.AxisListType.X, mybir.AluOpType.max)
            nc.vector.tensor_copy(am2[:, 1:2], red[:, 4:5])
            # cross-partition: result lands on every partition
            amg_loc = pool.tile([128, 2], F32)
            nc.gpsimd.partition_all_reduce(amg_loc[:], am2[:], 128,
                                           bass_isa.ReduceOp.max)

        # ---- global amax via AllReduce(max) on [128,2]
        cc_in = dram.tile([128, 2], F32)
        cc_out = dram.tile([128, 2], F32)
        ag = pool.tile([128, 2], F32)
        with nc.named_scope("cc_amax"):
            nc.sync.dma_start(cc_in[:], amg_loc[:])
            nc.gpsimd.collective_compute(
                "AllReduce", mybir.AluOpType.max,
                replica_groups=[list(range(N_CORES))],
                ins=[cc_in[:].opt()], outs=[cc_out[:].opt()])
            nc.sync.dma_start(ag[:], cc_out[:])

        # ---- scales (every partition computes the same values)
        scal = pool.tile([128, 8], F32)  # [s_x, s_w, rx, rw, sc4, ...]
        with nc.named_scope("scales"):
            nc.vector.tensor_scalar(scal[:, 0:1], ag[:, 0:1], RC, None,
                                    mybir.AluOpType.mult)       # s_x
            mx = pool.tile([128, 1], F32)
            nc.vector.tensor_tensor(mx[:], ag[:, 0:1], ag[:, 1:2],
                                    mybir.AluOpType.max)
            nc.vector.tensor_scalar(scal[:, 1:2], mx[:], RC, None,
                                    mybir.AluOpType.mult)       # s_w
            d2x = pool.tile([128, 2], F32)
            nc.vector.tensor_scalar(d2x[:, 0:1], scal[:, 0:1], 2.0, None,
                                    mybir.AluOpType.mult)
            nc.vector.tensor_scalar(d2x[:, 1:2], scal[:, 1:2], 2.0, None,
                                    mybir.AluOpType.mult)
            nc.vector.reciprocal(scal[:, 2:4], d2x[:])          # rx, rw
            ss = pool.tile([128, 1], F32)
            nc.vector.tensor_tensor(ss[:], scal[:, 0:1], scal[:, 1:2],
                                    mybir.AluOpType.mult)
            nc.vector.tensor_scalar(scal[:, 4:5], ss[:], 4.0, None,
                                    mybir.AluOpType.mult)       # 4*s_x*s_w

        # ---- HAM warm-up: ~4us of junk matmuls gated on the collective
        # result, so the PE leaves its cold 1.2GHz state while quant runs
        warm_lhs = pool.tile([128, 8], F8)
        warm_rhs = pool.tile([128, 512], F8)
        nc.vector.memset(warm_rhs[:], 0.0)
        nc.vector.memset(warm_lhs[:], 0.0)
        nc.vector.tensor_copy(warm_lhs[:, 0:2], ag[:, 0:2])
        warm_ps = psum.tile([128, 512], F32, name="ps")
        for _ in range(20):
            nc.tensor.matmul(warm_ps[0:8, :], warm_lhs[:], warm_rhs[:],
                             start=True, stop=True)

        # ---- bias broadcast: stage into wsl_sb row 0 (dead after amax pass)
        nc.sync.dma_start(wsl_sb[0:1, 0:DOUT], bias_d[:])
        nc.gpsimd.partition_broadcast(bias_bc[:], wsl_sb[0:1, 0:DOUT])
        # release wsl's 16KB/partition so the w streaming pool can use it
        wslctx.close()
        wpool = ctx.enter_context(tc.tile_pool(name="wpool", bufs=WT_BUFS))

        # ---- stream w (transposed) k-tiles; quantize x and w
        xq = pool.tile([128, KT, TSH], F8)
        wq = pool.tile([128, KT, DOUT], F8)
        wt_view = wt_d[:].rearrange("(k p) c -> p k c", p=128)
        with nc.named_scope("quant"):
            for k in range(KT):
                wt_t = wpool.tile([128, DOUT], F32, name="wt_t")
                nc.sync.dma_start(wt_t[:], wt_view[:, k, :])
                nc.vector.tensor_scalar(xq[:, k, :], xt_sb[:, k, :],
                                        scal[:, 2:3], None,
                                        mybir.AluOpType.mult)
                nc.vector.tensor_scalar(wq[:, k, :], wt_t[:],
                                        scal[:, 3:4], None,
                                        mybir.AluOpType.mult)

        # ---- matmuls + output scale/bias
        # groups of one token-tile m x 4 n-tiles = 4 PSUM banks; with
        # bufs=8 two groups are in flight so bank recycling (STT drain)
        # never stalls the PE. n is innermost: 4 consecutive matmuls share
        # the same stationary tile.
        with nc.named_scope("mm"):
            for m in range(MT):
                ptiles = [psum.tile([128, 512], F32, name="ps")
                          for _ in range(NT)]
                if USE_DOUBLE_ROW:
                    for kk in range(KT // 2):
                        for n in range(NT):
                            nc.tensor.matmul(
                                ptiles[n][:],
                                xq[:, 2 * kk:2 * kk + 2,
                                   m * 128:(m + 1) * 128],
                                wq[:, 2 * kk:2 * kk + 2,
                                   n * 512:(n + 1) * 512],
                                start=(kk == 0), stop=(kk == KT // 2 - 1),
                                perf_mode=mybir.MatmulPerfMode.DoubleRow)
                else:
                    for kk in range(KT):
                        for n in range(NT):
                            nc.tensor.matmul(
                                ptiles[n][:],
                                xq[:, kk, m * 128:(m + 1) * 128],
                                wq[:, kk, n * 512:(n + 1) * 512],
                                start=(kk == 0), stop=(kk == KT - 1))
                for n in range(NT):
                    osb = opool.tile([128, 512], F32, name="osb")
                    nc.vector.scalar_tensor_tensor(
                        osb[:], ptiles[n][:], scal[:, 4:5],
                        bias_bc[:, n * 512:(n + 1) * 512],
                        mybir.AluOpType.mult, mybir.AluOpType.add)
                    nc.sync.dma_start(
                        out_d[m * 128:(m + 1) * 128,
                              n * 512:(n + 1) * 512], osb[:])

    nc.compile()
    _built = nc
    return nc


def kernel(x, weight, bias):
    x = np.asarray(x, dtype=np.float32)
    weight = np.asarray(weight, dtype=np.float32)
    bias = np.asarray(bias, dtype=np.float32)
    x2 = np.ascontiguousarray(x.reshape(TOK, DIN))
    wt = np.ascontiguousarray(weight.T)                    # [DIN, DOUT]
    rows = DOUT // N_CORES                                 # 256
    in_maps = []
    for i in range(N_CORES):
        in_maps.append({
            "xt": np.ascontiguousarray(x2[i * TSH:(i + 1) * TSH].T),
            "wt": wt,
            "wsl": np.ascontiguousarray(
                weight[i * rows:(i + 1) * rows]).reshape(128, -1),
            "bias": np.ascontiguousarray(bias.reshape(1, DOUT)),
        })
    nc = _build()
    br = run_bass_kernel_spmd(nc, in_maps, list(range(N_CORES)))
    out = np.concatenate([r["out"] for r in br.results], axis=0)
    return np.ascontiguousarray(out.reshape(B, S, DOUT))


# revision 21
# speedup vs baseline: 1.0151x; 1.0151x over previous
"""FP8Linear on 8 Trainium2 NeuronCores (Bass/Tile, SPMD).

Reference math (per nn_FP8Linear):
    amax_x = max|x|, amax_w = max|w|               (global)
    x_scale = amax_x / C,  w_scale = max(amax_x, amax_w) / C,  C = fp32(448*0.8)
    out = dequant(e4m3fn(x/x_scale)) @ dequant(e4m3fn(w/w_scale)).T + bias

Sharding: tokens (B*S=8192) split across 8 cores (1024 each); weight
replicated. Inputs are staged pre-transposed ([Din, tok] / [Din, Dout]) so the
contraction dim lands on SBUF partitions with no on-device transposes.

Device per core:
  - local abs-max of its x shard + its 1/8 row-slice of w (extra 2MB input)
  - AllReduce(max) of [amax_x, amax_w] across the 8 cores (ncfw; a dummy
    warm-up collective is issued at t=0 to absorb channel-setup latency)
  - scales: s = amax*(1/C); quantize t = x * rn(1/(2s)) -> TRN fp8e4.
    TRN fp8e4 max is +-240 (not e4m3fn's 448); quantizing x/(2s) instead of
    x/s keeps values <= 179.2 and exactly halves the e4m3fn grid, which the
    final output scale 4*s_x*s_w undoes.
  - fp8 matmuls accumulating over Din in PSUM; out = psum*(4 s_x s_w) + bias
"""
import numpy as np
from contextlib import ExitStack

import concourse.bacc as bacc
import concourse.bass as bass
import concourse.bass_isa as bass_isa
import concourse.tile as tile
from concourse import mybir
from concourse.bass_utils import run_bass_kernel_spmd

N_CORES = 8
B, S, DIN, DOUT = 4, 2048, 2048, 2048
TOK = B * S                  # 8192 tokens
TSH = TOK // N_CORES         # 1024 tokens per core
KT = DIN // 128              # 16 k-tiles
NT = DOUT // 512             # 4 n-groups
MT = TSH // 128              # 8 token-tiles

C_F64 = 448.0 * 0.8          # what jax sees before fp32 demotion
RC = float(np.float32(1.0 / np.float64(np.float32(C_F64))))  # rn(1/C)

USE_DOUBLE_ROW = True        # fp8 DoubleRow perf mode (2x PE, ~1e-4 extra err)
import os as _os
USE_CC_WARM = _os.environ.get("CC_WARM", "1") == "1"
WT_BUFS = 9                  # fp32 w-tile streaming slots (8KB/partition each)
ENABLE_LDW_OPT = False       # walrus rejects ldw-opt on bass-emitted Ldweights


def _patch_ldw_opt():
    """The walrus cmdline in bass_utils hardcodes --enable-ldw-opt=false;
    rewrite it so repeated-stationary matmuls skip the weight reload."""
    import concourse.bass_utils as bu
    if getattr(bu, "_ldw_opt_patched", False):
        return
    orig = bu.run_command

    def patched(cmd, *a, **kw):
        if isinstance(cmd, list):
            cmd = ["--enable-ldw-opt=true" if c == "--enable-ldw-opt=false"
                   else c for c in cmd]
        return orig(cmd, *a, **kw)

    bu.run_command = patched
    bu._ldw_opt_patched = True


if ENABLE_LDW_OPT:
    _patch_ldw_opt()

F32 = mybir.dt.float32
F8 = mybir.dt.float8e4

_built = None


def _build():
    global _built
    if _built is not None:
        return _built
    nc = bacc.Bacc("TRN2", target_bir_lowering=False, num_devices=N_CORES)

    xt_d = nc.dram_tensor("xt", [DIN, TSH], F32, kind="ExternalInput")
    wt_d = nc.dram_tensor("wt", [DIN, DOUT], F32, kind="ExternalInput")
    wsl_d = nc.dram_tensor("wsl", [128, DIN * DOUT // N_CORES // 128], F32,
                           kind="ExternalInput")   # [128, 4096]
    bias_d = nc.dram_tensor("bias", [1, DOUT], F32, kind="ExternalInput")
    out_d = nc.dram_tensor("out", [TSH, DOUT], F32, kind="ExternalOutput")

    WSLF = DIN * DOUT // N_CORES // 128  # 4096

    with tile.TileContext(nc) as tc, ExitStack() as ctx:
        pool = ctx.enter_context(tc.tile_pool(name="pool", bufs=1))
        opool = ctx.enter_context(tc.tile_pool(name="opool", bufs=4))
        psum = ctx.enter_context(tc.tile_pool(name="psum", bufs=8, space="PSUM"))
        dram = ctx.enter_context(tc.tile_pool(name="dram", bufs=1, space="DRAM"))
        wslctx = ExitStack()
        wslpool = wslctx.enter_context(tc.tile_pool(name="wslpool", bufs=1))

        # ---- dummy collective at t=0: starts ncfw channel setup early
        warm = pool.tile([1, 8], F32)
        nc.vector.memset(warm[:], 0.0)
        cw_in = dram.tile([1, 8], F32)
        cw_out = dram.tile([1, 8], F32)
        if USE_CC_WARM:
            with nc.named_scope("cc_warm"):
                nc.sync.dma_start(cw_in[:], warm[:])
                nc.gpsimd.collective_compute(
                    "AllReduce", mybir.AluOpType.max,
                    replica_groups=[list(range(N_CORES))],
                    ins=[cw_in[:].opt()], outs=[cw_out[:].opt()])

        # ---- input DMA: x shard (transposed) in 4 chunks, then wsl, bias
        xt_sb = pool.tile([128, KT, TSH], F32)
        xt_view = xt_d[:].rearrange("(k p) c -> p k c", p=128)
        with nc.named_scope("dma_x"):
            for c in range(4):
                nc.sync.dma_start(xt_sb[:, 4 * c:4 * c + 4, :],
                                  xt_view[:, 4 * c:4 * c + 4, :])
        wsl_sb = wslpool.tile([128, WSLF], F32)
        bias_bc = pool.tile([128, DOUT], F32)
        with nc.named_scope("dma_wsl"):
            nc.sync.dma_start(wsl_sb[:], wsl_d[:])

        # ---- local abs-max
        red = pool.tile([128, 8], F32)
        nc.vector.memset(red[:], 0.0)
        with nc.named_scope("amax"):
            for c in range(4):
                nc.vector.tensor_reduce(red[:, c:c + 1],
                                        xt_sb[:, 4 * c:4 * c + 4, :],
                                        mybir.AxisListType.XY,
                                        mybir.AluOpType.max,
                                        apply_absolute_value=True)
            nc.vector.tensor_reduce(red[:, 4:5], wsl_sb[:],
                                    mybir.AxisListType.X,
                                    mybir.AluOpType.max,
                                    apply_absolute_value=True)
            am2 = pool.tile([128, 2], F32)
            nc.vector.tensor_reduce(am2[:, 0:1], red[:, 0:4],
                                    mybir.AxisListType.X, mybir.AluOpType.max)
            nc.vector.tensor_copy(am2[:, 1:2], red[:, 4:5])
            # cross-partition: result lands on every partition
            amg_loc = pool.tile([128, 2], F32)
            nc.gpsimd.partition_all_reduce(amg_loc[:], am2[:], 128,
                                           bass_isa.ReduceOp.max)

        # ---- global amax via AllReduce(max) on [128,2]
        cc_in = dram.tile([128, 2], F32)
        cc_out = dram.tile([128, 2], F32)
        ag = pool.tile([128, 2], F32)
        with nc.named_scope("cc_amax"):
            nc.sync.dma_start(cc_in[:], amg_loc[:])
            nc.gpsimd.collective_compute(
                "AllReduce", mybir.AluOpType.max,
                replica_groups=[list(range(N_CORES))],
                ins=[cc_in[:].opt()], outs=[cc_out[:].opt()])
            nc.sync.dma_start(ag[:], cc_out[:])

        # ---- scales (every partition computes the same values)
        scal = pool.tile([128, 8], F32)  # [s_x, s_w, rx, rw, sc4, ...]
        with nc.named_scope("scales"):
            nc.vector.tensor_scalar(scal[:, 0:1], ag[:, 0:1], RC, None,
                                    mybir.AluOpType.mult)       # s_x
            mx = pool.tile([128, 1], F32)
            nc.vector.tensor_tensor(mx[:], ag[:, 0:1], ag[:, 1:2],
                                    mybir.AluOpType.max)
            nc.vector.tensor_scalar(scal[:, 1:2], mx[:], RC, None,
                                    mybir.AluOpType.mult)       # s_w
            d2x = pool.tile([128, 2], F32)
            nc.vector.tensor_scalar(d2x[:, 0:1], scal[:, 0:1], 2.0, None,
                                    mybir.AluOpType.mult)
            nc.vector.tensor_scalar(d2x[:, 1:2], scal[:, 1:2], 2.0, None,
                                    mybir.AluOpType.mult)
            nc.vector.reciprocal(scal[:, 2:4], d2x[:])          # rx, rw
            ss = pool.tile([128, 1], F32)
            nc.vector.tensor_tensor(ss[:], scal[:, 0:1], scal[:, 1:2],
                                    mybir.AluOpType.mult)
            nc.vector.tensor_scalar(scal[:, 4:5], ss[:], 4.0, None,
                                    mybir.AluOpType.mult)       # 4*s_x*s_w

        # ---- HAM warm-up: ~4us of junk matmuls gated on the collective
        # result, so the PE leaves its cold 1.2GHz state while quant runs
        warm_lhs = pool.tile([128, 8], F8)
        warm_rhs = pool.tile([128, 512], F8)
        nc.vector.memset(warm_rhs[:], 0.0)
        nc.vector.memset(warm_lhs[:], 0.0)
        nc.vector.tensor_copy(warm_lhs[:, 0:2], ag[:, 0:2])
        warm_ps = psum.tile([128, 512], F32, name="ps")
        for _ in range(20):
            nc.tensor.matmul(warm_ps[0:8, :], warm_lhs[:], warm_rhs[:],
                             start=True, stop=True)

        # ---- bias broadcast: stage into wsl_sb row 0 (dead after amax pass)
        nc.sync.dma_start(wsl_sb[0:1, 0:DOUT], bias_d[:])
        nc.gpsimd.partition_broadcast(bias_bc[:], wsl_sb[0:1, 0:DOUT])
        # release wsl's 16KB/partition so the w streaming pool can use it
        wslctx.close()
        wpool = ctx.enter_context(tc.tile_pool(name="wpool", bufs=WT_BUFS))

        # ---- stream w (transposed) k-tiles; quantize x and w
        xq = pool.tile([128, KT, TSH], F8)
        wq = pool.tile([128, KT, DOUT], F8)
        wt_view = wt_d[:].rearrange("(k p) c -> p k c", p=128)
        with nc.named_scope("quant"):
            for k in range(KT):
                wt_t = wpool.tile([128, DOUT], F32, name="wt_t")
                nc.sync.dma_start(wt_t[:], wt_view[:, k, :])
                nc.vector.tensor_scalar(xq[:, k, :], xt_sb[:, k, :],
                                        scal[:, 2:3], None,
                                        mybir.AluOpType.mult)
                nc.vector.tensor_scalar(wq[:, k, :], wt_t[:],
                                        scal[:, 3:4], None,
                                        mybir.AluOpType.mult)

        # ---- matmuls + output scale/bias
        # groups of one token-tile m x 4 n-tiles = 4 PSUM banks; with
        # bufs=8 two groups are in flight so bank recycling (STT drain)
        # never stalls the PE. n is innermost: 4 consecutive matmuls share
        # the same stationary tile.
        with nc.named_scope("mm"):
            for m in range(MT):
                ptiles = [psum.tile([128, 512], F32, name="ps")
                          for _ in range(NT)]
                if USE_DOUBLE_ROW:
                    for kk in range(KT // 2):
                        for n in range(NT):
                            nc.tensor.matmul(
                                ptiles[n][:],
                                xq[:, 2 * kk:2 * kk + 2,
                                   m * 128:(m + 1) * 128],
                                wq[:, 2 * kk:2 * kk + 2,
                                   n * 512:(n + 1) * 512],
                                start=(kk == 0), stop=(kk == KT // 2 - 1),
                                perf_mode=mybir.MatmulPerfMode.DoubleRow)
                else:
                    for kk in range(KT):
                        for n in range(NT):
                            nc.tensor.matmul(
                                ptiles[n][:],
                                xq[:, kk, m * 128:(m + 1) * 128],
                                wq[:, kk, n * 512:(n + 1) * 512],
                                start=(kk == 0), stop=(kk == KT - 1))
                for n in range(NT):
                    osb = opool.tile([128, 512], F32, name="osb")
                    nc.vector.scalar_tensor_tensor(
                        osb[:], ptiles[n][:], scal[:, 4:5],
                        bias_bc[:, n * 512:(n + 1) * 512],
                        mybir.AluOpType.mult, mybir.AluOpType.add)
                    nc.sync.dma_start(
                        out_d[m * 128:(m + 1) * 128,
                              n * 512:(n + 1) * 512], osb[:])

    nc.compile()
    _built = nc
    return nc


def kernel(x, weight, bias):
    x = np.asarray(x, dtype=np.float32)
    weight = np.asarray(weight, dtype=np.float32)
    bias = np.asarray(bias, dtype=np.float32)
    x2 = np.ascontiguousarray(x.reshape(TOK, DIN))
    wt = np.ascontiguousarray(weight.T)                    # [DIN, DOUT]
    rows = DOUT // N_CORES                                 # 256
    in_maps = []
    for i in range(N_CORES):
        in_maps.append({
            "xt": np.ascontiguousarray(x2[i * TSH:(i + 1) * TSH].T),
            "wt": wt,
            "wsl": np.ascontiguousarray(
                weight[i * rows:(i + 1) * rows]).reshape(128, -1),
            "bias": np.ascontiguousarray(bias.reshape(1, DOUT)),
        })
    nc = _build()
    br = run_bass_kernel_spmd(nc, in_maps, list(range(N_CORES)))
    out = np.concatenate([r["out"] for r in br.results], axis=0)
    return np.ascontiguousarray(out.reshape(B, S, DOUT))
